# revision 21
# baseline (speedup 1.0000x reference)
"""Trainium2 Bass kernel for the pairwise concordance-index loss.

reference:
    loss = sum_{i<j, f_i=f_j=1} relu((p_i-p_j)(t_i-t_j)) / 100 / n_pairs

Math:
  Work only on the COMPACTED set (rows with f=1, m of them).
  M[i,j] = (p_i-p_j)(t_i-t_j) = A^T B, rank 4:
      A = [u, 1, p, t], B = [1, u, -t, -p], u = p*t.
  sum relu(M) = 0.5*(sum M + sum |M|); sum M has an O(m) closed form done
  on the host in fp64 over the same bf16 values the device uses; sum |M|
  is the O(m^2) part done on device.

Device decomposition (8 cores, identical program, data-sharded):
  Nb = ceil(m/128) row-blocks of 128; core k owns nb = ceil(Nb/8)
  consecutive block slots (slots past Nb hold zeros).  Each block
  processes its cyclic column window (offsets e wrapping mod Nb*128);
  the e=0 slab (and for even Nb the antipodal slab) is pre-scaled 0.5 on
  the host so all device sums have uniform weight.
  The core's col stream (nb*C cols) is split into 4 per-quad streams
  with ENGINE AFFINITY: quads 0,1 feed the DVE, quads 2,3 feed the
  ScalarE, stream lengths proportional to engine speed.  Each quad owns
  one persistent [128,2,512] PSUM tile (2 banks) and ping-pongs
  fill -> abs-row-sum -> refill; the two quads per engine are staggered
  so the engine never idles.  Quads run as concurrent K=4 bf16 matmuls
  packed into disjoint 32-row PE groups via tile_position.  At the end a
  K=128 fp32 ones-matmul folds acc [128,NJ] -> [1,NJ], copied and DMA'd
  out as a single line.
  Input arrives as per-quad [4,W] DMAs (2 column chunks) spread over the
  GpSimd and Sync queues.
"""

import numpy as np

B = 8192
P = 128
NCORE = 8

_cache = {}


def _plan(m):
    """Geometry from compacted count m."""
    Nb = -(-m // P)               # real cyclic block count
    nb = -(-Nb // NCORE)          # block slots per core
    if Nb % 2 == 0:
        C = (Nb // 2 + 1) * P     # offsets e=0..Nb/2; e=0, e=Nb/2 at 0.5
    else:
        C = ((Nb + 1) // 2) * P   # offsets e=0..(Nb-1)/2; e=0 at 0.5
    tot = nb * C                  # cols per core (even: C mult of 128)
    # ACT-side quad stream: multiple of 1024 near speed-proportional share
    qa = int(tot // 2 * 0.474) // 1024 * 1024
    qa = max(1024, qa)
    qd = tot // 2 - qa            # DVE-side quad stream
    WA = 3 * P                    # stationary slab area (3 block slots)
    W = WA + qd                   # line length (A-quad lines zero-padded)
    return Nb, nb, C, (qd, qd, qa, qa), WA, W


def _jobs_of(L):
    """Job sizes for a quad stream of L cols (full 1024s + remainder)."""
    out = [1024] * (L // 1024)
    if L % 1024:
        out.append(L % 1024)
    return out


def _quad_offs(qlen):
    offs = [0]
    for L in qlen:
        offs.append(offs[-1] + L)
    return offs


def _quad_slots(q, qlen, C):
    """Distinct local block slots touched by quad q's stream, in order."""
    offs = _quad_offs(qlen)
    lo, hi = offs[q], offs[q + 1]
    return list(range(lo // C, (hi - 1) // C + 1))


def _segments(q, qlen, C):
    """Per job: list of (tile_col0, ncols, slot, streampos) matmul segs.

    tile_col0 is the column offset within the quad's [P, 1024] tile
    (bank = tile_col0 // 512); streampos indexes the quad's b-stream.
    """
    offs = _quad_offs(qlen)
    L = qlen[q]
    slots = _quad_slots(q, qlen, C)
    jobs = []
    for j0 in range(0, L, 1024):
        j1 = min(j0 + 1024, L)
        segs = []
        c = j0
        while c < j1:
            nxt = min(j1, (c // 512 + 1) * 512)       # bank boundary
            gpos = offs[q] + c
            blk = gpos // C
            nxt = min(nxt, (blk + 1) * C - offs[q])   # block boundary
            segs.append((c - j0, nxt - c, slots.index(blk), c))
            c = nxt
        jobs.append(segs)
    return jobs


def _build(m):
    """Build + compile the Bass module for compacted size m."""
    import concourse.bacc as bacc
    import concourse.tile as tile
    import concourse.mybir as mybir

    f32 = mybir.dt.float32
    bf16 = mybir.dt.bfloat16
    Nb, nb, C, qlen, WA, W = _plan(m)
    jobs_q = [_jobs_of(L) for L in qlen]
    segs_q = [_segments(q, qlen, C) for q in range(4)]
    NJ = sum(len(j) for j in jobs_q)
    nround = max(len(j) for j in jobs_q)

    nc = bacc.Bacc("TRN2", target_bir_lowering=False, debug=False,
                   num_devices=NCORE)
    in_dram = nc.dram_tensor("inp", [4, 4, W], bf16, kind="ExternalInput")
    out_dram = nc.dram_tensor("acc", [1, NJ], f32, kind="ExternalOutput")

    with tile.TileContext(nc) as tc:
        with (
            tc.tile_pool(name="inp", bufs=1) as inp_pool,
            tc.tile_pool(name="ps", bufs=1, space="PSUM") as ps,
        ):
            sb = inp_pool.tile([P, W], bf16)
            dma_eng = [nc.gpsimd, nc.sync]
            cuts = (0, min(WA + 1024, W), min(WA + 2048, W), W)
            for ci in range(3):
                c0, c1 = cuts[ci], cuts[ci + 1]
                if c0 >= c1:
                    continue
                for q in range(4):
                    dma_eng[q % 2].dma_start(
                        sb[32 * q:32 * q + 4, c0:c1], in_dram.ap()[q, :, c0:c1])

            acc = inp_pool.tile([P, NJ], f32)
            fold_sb = inp_pool.tile([1, NJ], f32)

            tiles = [ps.tile([P, 2, 512], f32, name=f"qt{q}")
                     for q in range(4)]
            fold_ps = tiles[0][0:1, 0:1, 0:NJ]

            job = 0
            for r in range(nround):
                for q in range(4):
                    if r >= len(jobs_q[q]):
                        continue
                    jlen = jobs_q[q][r]
                    t = tiles[q]
                    for (c0, n, slot, spos) in segs_q[q][r]:
                        nc.tensor.matmul(
                            t[:, c0 // 512, c0 % 512:c0 % 512 + n],
                            sb[32 * q:32 * q + 4, P * slot:P * slot + P],
                            sb[32 * q:32 * q + 4, WA + spos:WA + spos + n],
                            start=True, stop=True,
                            tile_position=(32 * q, 0),
                        )
                    if jlen == 1024:
                        red = t[:, :, :]
                    elif jlen > 512:
                        red = t[:, :, :].rearrange("p a b -> p (a b)")[:, 0:jlen]
                    else:
                        red = t[:, 0, 0:jlen]
                    if q < 2:
                        nc.vector.tensor_reduce(
                            acc[:, job:job + 1], red,
                            axis=mybir.AxisListType.XY
                            if red.ndim == 3 else mybir.AxisListType.X,
                            op=mybir.AluOpType.add,
                            apply_absolute_value=True,
                        )
                    else:
                        nc.scalar.activation(
                            red, red, mybir.ActivationFunctionType.Abs,
                            accum_out=acc[:, job:job + 1],
                        )
                    job += 1

            assert job == NJ, job
            ones = nc.const_aps.tensor(1.0, [P, 1], f32)
            nc.tensor.matmul(fold_ps, ones, acc[:, :],
                             start=True, stop=True)
            nc.scalar.copy(fold_sb[:, :], fold_ps[:, :])
            nc.gpsimd.dma_start(out_dram.ap()[:, :], fold_sb[:, :])

    nc.compile()
    return nc


def _get_nc(m=None):
    if m is None:
        m = _cache["last_m"]
    key = _plan(m)[:3]
    if ("nc", key) not in _cache:
        _cache[("nc", key)] = _build(m)
    _cache["last_m"] = m
    return _cache[("nc", key)]


def _make_in_maps(p, t, f, u):
    """Compact by flag, build per-core [4,4,W] bf16 input arrays.

    Returns (in_maps, A_pad, B_pad) with the padded bf16 factor arrays the
    host closed form must use.
    """
    import ml_dtypes

    idx = np.nonzero(f != 0.0)[0]
    m = len(idx)
    Nb, nb, C, qlen, WA, W = _plan(m)
    offs = _quad_offs(qlen)
    m_cyc = Nb * P                # cyclic window modulus
    m_pad = NCORE * nb * P        # block-slot span (>= m_cyc)

    one = np.ones(m, np.float32)
    bf = ml_dtypes.bfloat16
    A = np.zeros((4, m_pad), dtype=bf)
    Bm = np.zeros((4, m_pad), dtype=bf)
    A[:, :m] = np.stack([u[idx], one, p[idx], t[idx]]).astype(bf)
    Bm[:, :m] = np.stack([one, u[idx], -t[idx], -p[idx]]).astype(bf)
    B32 = Bm.astype(np.float32)

    in_maps = []
    for k in range(NCORE):
        arr = np.zeros((4, 4, W), dtype=bf)
        stream = np.zeros((4, nb * C), dtype=bf)
        for l in range(nb):
            a = nb * k + l
            if a >= Nb:
                continue          # dummy slot, stays zero
            cols = (P * a + np.arange(C)) % m_cyc
            blockw = B32[:, cols].copy()
            blockw[:, :P] *= 0.5
            if Nb % 2 == 0:
                blockw[:, C - P:] *= 0.5
            stream[:, C * l:C * (l + 1)] = blockw.astype(bf)
        for q in range(4):
            arr[q, :, WA:WA + qlen[q]] = stream[:, offs[q]:offs[q + 1]]
            for si, blk in enumerate(_quad_slots(q, qlen, C)):
                a = nb * k + blk
                arr[q, :, P * si:P * si + P] = A[:, P * a:P * a + P]
        in_maps.append({"inp": arr})
    _cache["last_m"] = m
    return in_maps, A, Bm


def kernel(pred, gt, gt_fracTime, gt_ifMOF):
    from concourse import bass_utils

    pred = np.asarray(pred)
    gt = np.asarray(gt)
    ift = int(np.asarray(gt_fracTime))
    imf = int(np.asarray(gt_ifMOF))

    p = pred.astype(np.float32)
    t = gt[:, ift].astype(np.float32)
    f = (gt[:, imf] == 1).astype(np.float32)
    u = (p * t).astype(np.float32)

    in_maps, A, Bm = _make_in_maps(p, t, f, u)
    nc = _get_nc()
    res = bass_utils.run_bass_kernel_spmd(nc, in_maps, core_ids=list(range(NCORE)))

    # T = sum_{i<j} |M| (uniform weight; fold already summed partitions)
    T = 0.0
    for r in res.results:
        T += r["acc"].astype(np.float64).sum()

    # host closed form in fp64 over the same bf16 values the device used:
    # sum_{i<j} M = (sum_{i,j} M - sum_diag M) / 2
    A64 = A.astype(np.float64)
    B64 = Bm.astype(np.float64)
    S_all = (A64.sum(axis=1) * B64.sum(axis=1)).sum()
    D_diag = (A64 * B64).sum()
    S_half = (S_all - D_diag) / 2.0

    m = float(len(np.nonzero(f != 0.0)[0]))
    n_pairs = (m * m - m) / 2.0

    loss = 0.5 * (S_half + T) / 100.0 / n_pairs
    return np.asarray(np.float32(loss))


# revision 22
# speedup vs baseline: 1.0040x; 1.0040x over previous
"""Trainium2 Bass kernel for the pairwise concordance-index loss.

reference:
    loss = sum_{i<j, f_i=f_j=1} relu((p_i-p_j)(t_i-t_j)) / 100 / n_pairs

Math:
  Work only on the COMPACTED set (rows with f=1, m of them).
  M[i,j] = (p_i-p_j)(t_i-t_j) = A^T B, rank 4:
      A = [u, 1, p, t], B = [1, u, -t, -p], u = p*t.
  sum relu(M) = 0.5*(sum M + sum |M|); sum M has an O(m) closed form done
  on the host in fp64 over the same bf16 values the device uses; sum |M|
  is the O(m^2) part done on device.

Device decomposition (8 cores, identical program, data-sharded):
  Nb = ceil(m/128) row-blocks of 128; core k owns nb = ceil(Nb/8)
  consecutive block slots (slots past Nb hold zeros).  Each block
  processes its cyclic column window (offsets e wrapping mod Nb*128);
  the e=0 slab (and for even Nb the antipodal slab) is pre-scaled 0.5 on
  the host so all device sums have uniform weight.
  The core's col stream (nb*C cols) is split into 4 per-quad streams
  with ENGINE AFFINITY: quads 0,1 feed the DVE, quads 2,3 feed the
  ScalarE, stream lengths proportional to engine speed.  Each quad owns
  one persistent [128,2,512] PSUM tile (2 banks) and ping-pongs
  fill -> abs-row-sum -> refill; the two quads per engine are staggered
  so the engine never idles.  Quads run as concurrent K=4 bf16 matmuls
  packed into disjoint 32-row PE groups via tile_position.  At the end a
  K=128 fp32 ones-matmul folds acc [128,NJ] -> [1,NJ], copied and DMA'd
  out as a single line.
  Input arrives as per-quad [4,W] DMAs (2 column chunks) spread over the
  GpSimd and Sync queues.
"""

import numpy as np

B = 8192
P = 128
NCORE = 8

_cache = {}


def _plan(m):
    """Geometry from compacted count m."""
    Nb = -(-m // P)               # real cyclic block count
    nb = -(-Nb // NCORE)          # block slots per core
    if Nb % 2 == 0:
        C = (Nb // 2 + 1) * P     # offsets e=0..Nb/2; e=0, e=Nb/2 at 0.5
    else:
        C = ((Nb + 1) // 2) * P   # offsets e=0..(Nb-1)/2; e=0 at 0.5
    tot = nb * C                  # cols per core (even: C mult of 128)
    # ACT-side quad stream: multiple of 1024 near speed-proportional share
    qa = int(tot // 2 * 0.474) // 1024 * 1024
    qa = max(1024, qa)
    qd = tot // 2 - qa            # DVE-side quad stream
    WA = 3 * P                    # stationary slab area (3 block slots)
    W = WA + qd                   # line length (A-quad lines zero-padded)
    return Nb, nb, C, (qd, qd, qa, qa), WA, W


def _jobs_of(L):
    """Job sizes for a quad stream of L cols (full 1024s + remainder)."""
    out = [1024] * (L // 1024)
    if L % 1024:
        out.append(L % 1024)
    return out


def _quad_offs(qlen):
    offs = [0]
    for L in qlen:
        offs.append(offs[-1] + L)
    return offs


def _quad_slots(q, qlen, C):
    """Distinct local block slots touched by quad q's stream, in order."""
    offs = _quad_offs(qlen)
    lo, hi = offs[q], offs[q + 1]
    return list(range(lo // C, (hi - 1) // C + 1))


def _segments(q, qlen, C):
    """Per job: list of (tile_col0, ncols, slot, streampos) matmul segs.

    tile_col0 is the column offset within the quad's [P, 1024] tile
    (bank = tile_col0 // 512); streampos indexes the quad's b-stream.
    """
    offs = _quad_offs(qlen)
    L = qlen[q]
    slots = _quad_slots(q, qlen, C)
    jobs = []
    for j0 in range(0, L, 1024):
        j1 = min(j0 + 1024, L)
        segs = []
        c = j0
        while c < j1:
            nxt = min(j1, (c // 512 + 1) * 512)       # bank boundary
            gpos = offs[q] + c
            blk = gpos // C
            nxt = min(nxt, (blk + 1) * C - offs[q])   # block boundary
            segs.append((c - j0, nxt - c, slots.index(blk), c))
            c = nxt
        jobs.append(segs)
    return jobs


def _build(m):
    """Build + compile the Bass module for compacted size m."""
    import concourse.bacc as bacc
    import concourse.tile as tile
    import concourse.mybir as mybir

    f32 = mybir.dt.float32
    bf16 = mybir.dt.bfloat16
    Nb, nb, C, qlen, WA, W = _plan(m)
    jobs_q = [_jobs_of(L) for L in qlen]
    segs_q = [_segments(q, qlen, C) for q in range(4)]
    NJ = sum(len(j) for j in jobs_q)
    nround = max(len(j) for j in jobs_q)

    nc = bacc.Bacc("TRN2", target_bir_lowering=False, debug=False,
                   num_devices=NCORE)
    in_dram = nc.dram_tensor("inp", [4, 4, W], bf16, kind="ExternalInput")
    out_dram = nc.dram_tensor("acc", [1, NJ], f32, kind="ExternalOutput")

    with tile.TileContext(nc) as tc:
        with (
            tc.tile_pool(name="inp", bufs=1) as inp_pool,
            tc.tile_pool(name="ps", bufs=1, space="PSUM") as ps,
        ):
            sb = inp_pool.tile([P, W], bf16)
            dma_eng = [nc.gpsimd, nc.sync]
            cuts = (0, min(WA + 1024, W), min(WA + 2048, W), W)
            for ci in range(3):
                c0, c1 = cuts[ci], cuts[ci + 1]
                if c0 >= c1:
                    continue
                for q in range(4):
                    dma_eng[q % 2].dma_start(
                        sb[32 * q:32 * q + 4, c0:c1], in_dram.ap()[q, :, c0:c1])

            acc = inp_pool.tile([P, NJ], f32)
            fold_sb = inp_pool.tile([1, NJ], f32)

            tiles = [ps.tile([P, 2, 512], f32, name=f"qt{q}")
                     for q in range(4)]
            fold_ps = tiles[0][0:1, 0:1, 0:NJ]

            job = 0
            for r in range(nround):
                for q in range(4):
                    if r >= len(jobs_q[q]):
                        continue
                    jlen = jobs_q[q][r]
                    t = tiles[q]
                    for (c0, n, slot, spos) in segs_q[q][r]:
                        nc.tensor.matmul(
                            t[:, c0 // 512, c0 % 512:c0 % 512 + n],
                            sb[32 * q:32 * q + 4, P * slot:P * slot + P],
                            sb[32 * q:32 * q + 4, WA + spos:WA + spos + n],
                            start=True, stop=True,
                            tile_position=(32 * q, 0),
                        )
                    if jlen == 1024:
                        red = t[:, :, :]
                    elif jlen > 512:
                        red = t[:, :, :].rearrange("p a b -> p (a b)")[:, 0:jlen]
                    else:
                        red = t[:, 0, 0:jlen]
                    if q < 2:
                        nc.vector.tensor_reduce(
                            acc[:, job:job + 1], red,
                            axis=mybir.AxisListType.XY
                            if red.ndim == 3 else mybir.AxisListType.X,
                            op=mybir.AluOpType.add,
                            apply_absolute_value=True,
                        )
                    else:
                        nc.scalar.activation(
                            red, red, mybir.ActivationFunctionType.Abs,
                            accum_out=acc[:, job:job + 1],
                        )
                    job += 1

            assert job == NJ, job
            ones = nc.const_aps.tensor(1.0, [P, 1], f32)
            nc.tensor.matmul(fold_ps, ones, acc[:, :],
                             start=True, stop=True)
            nc.scalar.copy(fold_sb[:, :], fold_ps[:, :])
            nc.gpsimd.dma_start(out_dram.ap()[:, :], fold_sb[:, :],
                                single_packet=True)

    nc.compile()
    return nc


def _get_nc(m=None):
    if m is None:
        m = _cache["last_m"]
    key = _plan(m)[:3]
    if ("nc", key) not in _cache:
        _cache[("nc", key)] = _build(m)
    _cache["last_m"] = m
    return _cache[("nc", key)]


def _make_in_maps(p, t, f, u):
    """Compact by flag, build per-core [4,4,W] bf16 input arrays.

    Returns (in_maps, A_pad, B_pad) with the padded bf16 factor arrays the
    host closed form must use.
    """
    import ml_dtypes

    idx = np.nonzero(f != 0.0)[0]
    m = len(idx)
    Nb, nb, C, qlen, WA, W = _plan(m)
    offs = _quad_offs(qlen)
    m_cyc = Nb * P                # cyclic window modulus
    m_pad = NCORE * nb * P        # block-slot span (>= m_cyc)

    one = np.ones(m, np.float32)
    bf = ml_dtypes.bfloat16
    A = np.zeros((4, m_pad), dtype=bf)
    Bm = np.zeros((4, m_pad), dtype=bf)
    A[:, :m] = np.stack([u[idx], one, p[idx], t[idx]]).astype(bf)
    Bm[:, :m] = np.stack([one, u[idx], -t[idx], -p[idx]]).astype(bf)
    B32 = Bm.astype(np.float32)

    in_maps = []
    for k in range(NCORE):
        arr = np.zeros((4, 4, W), dtype=bf)
        stream = np.zeros((4, nb * C), dtype=bf)
        for l in range(nb):
            a = nb * k + l
            if a >= Nb:
                continue          # dummy slot, stays zero
            cols = (P * a + np.arange(C)) % m_cyc
            blockw = B32[:, cols].copy()
            blockw[:, :P] *= 0.5
            if Nb % 2 == 0:
                blockw[:, C - P:] *= 0.5
            stream[:, C * l:C * (l + 1)] = blockw.astype(bf)
        for q in range(4):
            arr[q, :, WA:WA + qlen[q]] = stream[:, offs[q]:offs[q + 1]]
            for si, blk in enumerate(_quad_slots(q, qlen, C)):
                a = nb * k + blk
                arr[q, :, P * si:P * si + P] = A[:, P * a:P * a + P]
        in_maps.append({"inp": arr})
    _cache["last_m"] = m
    return in_maps, A, Bm


def kernel(pred, gt, gt_fracTime, gt_ifMOF):
    from concourse import bass_utils

    pred = np.asarray(pred)
    gt = np.asarray(gt)
    ift = int(np.asarray(gt_fracTime))
    imf = int(np.asarray(gt_ifMOF))

    p = pred.astype(np.float32)
    t = gt[:, ift].astype(np.float32)
    f = (gt[:, imf] == 1).astype(np.float32)
    u = (p * t).astype(np.float32)

    in_maps, A, Bm = _make_in_maps(p, t, f, u)
    nc = _get_nc()
    res = bass_utils.run_bass_kernel_spmd(nc, in_maps, core_ids=list(range(NCORE)))

    # T = sum_{i<j} |M| (uniform weight; fold already summed partitions)
    T = 0.0
    for r in res.results:
        T += r["acc"].astype(np.float64).sum()

    # host closed form in fp64 over the same bf16 values the device used:
    # sum_{i<j} M = (sum_{i,j} M - sum_diag M) / 2
    A64 = A.astype(np.float64)
    B64 = Bm.astype(np.float64)
    S_all = (A64.sum(axis=1) * B64.sum(axis=1)).sum()
    D_diag = (A64 * B64).sum()
    S_half = (S_all - D_diag) / 2.0

    m = float(len(np.nonzero(f != 0.0)[0]))
    n_pairs = (m * m - m) / 2.0

    loss = 0.5 * (S_half + T) / 100.0 / n_pairs
    return np.asarray(np.float32(loss))
